# revision 37
# baseline (speedup 1.0000x reference)
"""KVGather Trainium2 kernel.

Problem: out[n, i, k] = r_weight[n, i, k] * kv[n, r_idx[n, i, k]]
  r_idx:    (16, 64, 8)  int64, values in [0, 64)
  r_weight: (16, 64, 8)  float32
  kv:       (16, 64, 64, 128) float32
  out:      (16, 64, 8, 64, 128) float32

Strategy: data-parallel over batch n across 8 NeuronCores (2 batches/core).
Per core: ~2.3 MB in, 32 MB out -> HBM-write-bound.

The gather+scale is reformulated as a one-hot matmul so all device
addressing is static:
  - Host casts kv to bf16 (rel err ~2e-3, well within the 2e-2 gate) and
    packs BOTH of the core's batches into one [128, F] plane: partitions
    0..63 hold batch-0 regions, 64..127 hold batch-1 regions.
  - Host builds selection matrices S (bf16): a one at row r + 64*b
    routes any slot of either batch through one bf16 matmul into fp32
    PSUM.
  - DVE/ACT drain PSUM -> SBUF fused with the f32 weight multiply.
  - All DMA is HWDGE; output stores go on the sync ring.

Engine-level load balancing: the high-index SDMA engines (15, often
12/13) run 10-20% slower than the rest (measured repeatedly; known
TRN2 quirk, magnitude drifts run to run), and an n-partition DMA is
dealt in equal contiguous row-blocks to the largest-divisor-of-n <= 16
engines starting at engine 0 (measured: 128 -> 16 engines, 120 -> 15,
92 -> 4 (!), 72 -> 12, 56 -> 14, 28 -> 14, 64/96/16 -> 16). Splitting
2 of the 32 one-MB stores as [72]+[56] (engines 0-11 x6 rows + engines
0-13 x4) instead of [128] (all x8) gives engines 0-11: 260 rows,
12-13: 248, 14-15: 240 across the kernel - a mild discount of the
slow engines that measured best (swept 0/1/2/3/4 splits) against the
drifting engine-rate profiles, so no straggler gates the kernel.
"""

import sys

for _p in ("/opt/trn_rl_repo",):
    if _p not in sys.path:
        sys.path.insert(0, _p)

import numpy as np
import ml_dtypes

from concourse import bass, bacc, tile
from concourse import mybir
from concourse.bass_utils import run_bass_kernel_spmd

# Problem constants (hardcoded per contract)
N, P2, TOPK, W2, C_KV = 16, 64, 8, 64, 128
N_CORES = 8
B = N // N_CORES            # batches per core = 2
SLOTS = P2 * TOPK           # 512 output slots per batch
G = B * SLOTS               # 1024 global slots per core
F = W2 * C_KV               # 8192 elements per region
FC = 8                      # kv load split (first matmul gates on 0.25MB)
F_PER_FC = F // FC          # 1024
FS = 4                      # store split (2048 f32 = 8KB rows)
F_PER_FS = F // FS          # 2048
N_CHUNK = 8                 # 8 uniform chunks of 128 slots
# Stores at these (chunk*FS + fs) indices are split [72]+[56] to
# discount slow SDMA engines 12-15 (see module docstring); the rest
# are single [128] stores. 2 of 32 (tuned against measured engine-rate profiles).
SPLIT_STORES = {8, 24}

_cached = {}


def _build_program():
    """Build the (input-independent) Bass program once."""
    if "nc" in _cached:
        return _cached["nc"]

    bf16 = mybir.dt.bfloat16
    f32 = mybir.dt.float32

    nc = bacc.Bacc()

    # Packed kv plane: partition p in [0,64) = batch 0 region p; p in
    # [64,128) = batch 1 region p-64; free = region elems (f-chunked).
    kv_d = nc.dram_tensor("kvp", [128, FC, F_PER_FC], bf16, kind="ExternalInput")
    # Selection matrices: s_d[r, c, o] - column o of chunk c has a one
    # at row r + 64*b for the slot routed to column o (columns beyond
    # the chunk size are zero).
    s_d = nc.dram_tensor("sel", [128, N_CHUNK, 128], bf16, kind="ExternalInput")
    # w_d[o, c] = f32 weight of chunk c's column o.
    w_d = nc.dram_tensor("wgt", [128, N_CHUNK], f32, kind="ExternalInput")
    # Output, flattened over (batch, slot): [1024, F].
    out_d = nc.dram_tensor("out", [G, F], f32, kind="ExternalOutput")

    with tile.TileContext(nc) as tc:
        with (
            tc.tile_pool(name="const", bufs=1) as const_pool,
            tc.tile_pool(name="kv", bufs=1) as kv_pool,
            tc.tile_pool(name="stage", bufs=12) as stage_pool,
            tc.tile_pool(name="psum", bufs=4, space=bass.MemorySpace.PSUM) as psum_pool,
        ):
            # sel as two tiles so chunk 0's LDWEIGHTS gates on 32KB
            # (Tile tracks deps per tile, not per slice).
            s0_sb = const_pool.tile([128, 1, 128], bf16, tag="sel0")
            s1_sb = const_pool.tile([128, N_CHUNK - 1, 128], bf16, tag="sel1")
            w_sb = const_pool.tile([128, N_CHUNK], f32, tag="wgt")
            kv_sb = {}
            for fc in range(FC):
                tkv = kv_pool.tile([128, F_PER_FC], bf16, tag=f"kv{fc}")
                kv_sb[fc] = tkv
            # Interleave loads across both HWDGE rings; the tiles that
            # gate the first stores go first on each ring.
            nc.scalar.dma_start(out=s0_sb[:], in_=s_d[:, 0:1, :])
            nc.sync.dma_start(out=kv_sb[0][:], in_=kv_d[:, 0, :])
            nc.scalar.dma_start(out=kv_sb[1][:], in_=kv_d[:, 1, :])
            nc.sync.dma_start(out=w_sb[:], in_=w_d[:])
            nc.scalar.dma_start(out=s1_sb[:], in_=s_d[:, 1:, :])
            nc.sync.dma_start(out=kv_sb[2][:], in_=kv_d[:, 2, :])
            nc.scalar.dma_start(out=kv_sb[3][:], in_=kv_d[:, 3, :])
            nc.sync.dma_start(out=kv_sb[4][:], in_=kv_d[:, 4, :])
            nc.scalar.dma_start(out=kv_sb[5][:], in_=kv_d[:, 5, :])
            nc.sync.dma_start(out=kv_sb[6][:], in_=kv_d[:, 6, :])
            nc.scalar.dma_start(out=kv_sb[7][:], in_=kv_d[:, 7, :])

            def sel_ap(ch):
                return s0_sb[:, 0, :] if ch == 0 else s1_sb[:, ch - 1, :]

            th_i = 0
            for ch in range(N_CHUNK):
                base = ch * 128
                for fs in range(FS):
                    ramp = ch == 0 and fs == 0
                    if not ramp:
                        stage = stage_pool.tile(
                            [128, F_PER_FS], f32, tag="stage"
                        )
                    for th in range(F_PER_FS // 1024):
                        if ramp:
                            # Ramp: per-half stage tiles so each 0.5MB
                            # store issues right after its own drain
                            # (deps are tile-granular).
                            stage = stage_pool.tile(
                                [128, F_PER_FS], f32, tag="stage"
                            )
                        # Drains are PSUM-read-bound (~1.2us on either
                        # engine), so alternate DVE/ACT 1:1 per drain.
                        use_dve = th_i % 2 == 0
                        th_i += 1
                        # 2-bank PSUM tile; two 512-wide matmuls fill it,
                        # one 1024-wide op drains it.
                        ps = psum_pool.tile([128, 1024], f32, tag="ps")
                        for h in range(2):
                            f0 = fs * F_PER_FS + th * 1024 + h * 512
                            fc, off = divmod(f0, F_PER_FC)
                            nc.tensor.matmul(
                                ps[:, h * 512 : (h + 1) * 512],
                                sel_ap(ch),
                                kv_sb[fc][:, off : off + 512],
                                start=True,
                                stop=True,
                            )
                        sl = stage[:, 0:1024] if ramp else stage[
                            :, th * 1024 : (th + 1) * 1024
                        ]
                        if use_dve:
                            nc.vector.tensor_mul(
                                sl,
                                ps[:],
                                w_sb[:, ch : ch + 1].broadcast_to([128, 1024]),
                            )
                        else:
                            nc.scalar.activation(
                                sl,
                                ps[:],
                                mybir.ActivationFunctionType.Copy,
                                scale=w_sb[:, ch : ch + 1],
                            )
                        if ramp:
                            nc.sync.dma_start(
                                out=out_d[
                                    base : base + 128,
                                    th * 1024 : (th + 1) * 1024,
                                ],
                                in_=stage[:, 0:1024],
                            )
                    if ramp:
                        continue
                    fsl = slice(fs * F_PER_FS, (fs + 1) * F_PER_FS)
                    eng = nc.sync
                    if ch * FS + fs in SPLIT_STORES:
                        # [72] -> engines 0-11 (6 rows each);
                        # [56] -> engines 0-13 (4 rows each).
                        eng.dma_start(
                            out=out_d[base : base + 72, fsl],
                            in_=stage[0:72, :],
                        )
                        eng.dma_start(
                            out=out_d[base + 72 : base + 128, fsl],
                            in_=stage[72:128, :],
                        )
                    else:
                        # [128] -> all 16 engines, 8 rows each.
                        eng.dma_start(
                            out=out_d[base : base + 128, fsl],
                            in_=stage[:],
                        )

    nc.compile()
    _cached["nc"] = nc
    return nc


def _prep_inputs(r_idx, r_weight, kv):
    """Shard + transform host inputs into per-core in_maps."""
    r_idx = np.asarray(r_idx).astype(np.int64)
    r_weight = np.asarray(r_weight).astype(np.float32)
    kv = np.asarray(kv).astype(np.float32)

    kv_bf = kv.astype(ml_dtypes.bfloat16)

    in_maps = []
    for m in range(N_CORES):
        bsl = slice(m * B, (m + 1) * B)
        idx = r_idx[bsl].reshape(G)               # [1024] region ids
        wgt = r_weight[bsl].reshape(G)            # [1024] f32

        im = {}
        # [64, F] batch0 over [64, F] batch1 -> [128, F] -> [128, FC, F/FC]
        plane = np.concatenate(
            [
                kv_bf[m * B + 0].reshape(P2, F),
                kv_bf[m * B + 1].reshape(P2, F),
            ],
            axis=0,
        )
        im["kvp"] = np.ascontiguousarray(plane.reshape(128, FC, F_PER_FC))

        S = np.zeros((128, N_CHUNK, 128), dtype=ml_dtypes.bfloat16)
        W = np.zeros((128, N_CHUNK), dtype=np.float32)
        for ch in range(N_CHUNK):
            g = np.arange(ch * 128, (ch + 1) * 128)  # global slot ids
            b = g // SLOTS
            rows = idx[g] + 64 * b
            S[rows, ch, np.arange(128)] = 1.0
            W[:, ch] = wgt[g]
        im["sel"] = S
        im["wgt"] = W
        in_maps.append(im)
    return in_maps


def _ensure_ntff_hook():
    """The agent image's antenv lacks axon_hooks, so the boot-time NTFF
    hook registration silently no-ops. Recreate the module and register
    the ctypes hook so trace=True yields exec_time_ns."""
    import types
    import antenv

    if "antenv.axon_hooks" in sys.modules:
        return
    mod = types.ModuleType("antenv.axon_hooks")
    _state = {"hook": None}
    mod.set_axon_ntff_profile_hook = lambda h: _state.__setitem__("hook", h)
    mod.get_axon_ntff_profile_hook = lambda: _state["hook"]
    sys.modules["antenv.axon_hooks"] = mod
    antenv.axon_hooks = mod
    try:
        if "/root/.axon_site" not in sys.path:
            sys.path.insert(0, "/root/.axon_site")
        from trn_agent_boot.trn_boot import _ntff_profile_via_ctypes

        hook = _ntff_profile_via_ctypes("/opt/axon/libaxon_pjrt.so")
        if hook is not None:
            mod.set_axon_ntff_profile_hook(hook)
    except Exception:
        pass


def kernel(r_idx, r_weight, kv, _trace=False, _trace_kwargs=None):
    if _trace:
        _ensure_ntff_hook()
    nc = _build_program()
    in_maps = _prep_inputs(r_idx, r_weight, kv)
    res = run_bass_kernel_spmd(
        nc,
        in_maps,
        core_ids=list(range(N_CORES)),
        trace=_trace,
        **(_trace_kwargs or {}),
    )
    out = np.empty((N, P2, TOPK, W2, C_KV), dtype=np.float32)
    for m in range(N_CORES):
        o = res.results[m]["out"]  # [G, F]
        out[m * B : (m + 1) * B] = o.reshape(B, P2, TOPK, W2, C_KV)
    if _trace:
        return out, res
    return out


if __name__ == "__main__":
    rng = np.random.default_rng(0)
    r_idx = rng.integers(0, P2, (N, P2, TOPK)).astype(np.int64)
    r_weight = rng.random((N, P2, TOPK), dtype=np.float32)
    kv = rng.standard_normal((N, P2, W2, C_KV), dtype=np.float32)
    out = kernel(r_idx, r_weight, kv)
    # local reference
    bidx = np.arange(N)[:, None, None]
    exp = r_weight[..., None, None] * kv[bidx, r_idx]
    err = np.abs(out - exp).max() / (np.abs(exp).max() + 1e-30)
    print("abs-rel err:", err)
